# revision 26
# baseline (speedup 1.0000x reference)
"""Distributed gathered-matvec kernel for nn_CubicalModel_ISM.

Reference computes Xp = I @ p, Yp = J @ p (I, J: [784, 50000]) and then
gathers 100 entries of each via inds1/inds2. Only the gathered rows are
ever observed, so the kernel computes exactly those dot products on the
deduplicated index sets (u1, u2 unique rows; typically ~94 each):

    out[i] = I[row_i, :] @ p   for the u1 + u2 unique gather rows

Strategy (8 NeuronCores):
  - Host selects and dedups the needed rows of each matrix (the gather
    is host-side sharding) and casts them to bf16: ~8x less HBM traffic
    than the full 784-row matvec, 2x less than fp32.
  - Contraction dim P=50000 is sharded column-wise across 8 cores;
    each core's 6250 slice maps to 48 k-subtiles of 128 partitions
    plus one of 112 (106 real + 6 zero rows). Line counts divisible by
    16 matter: the HWDGE splits a DMA across SDMA engines only in
    equal line counts, so 128/112 lines -> all 16 engines (~390 GB/s)
    while 125 lines -> 5 engines (~115 GB/s, measured).
  - p keeps fp32-level precision via a bf16 hi + bf16 lo split; the
    PE computes [p_hi, p_lo]^T @ X per subtile into a [2, R] PSUM
    accumulator (cols = u1 I-rows | u2 J-rows), fp32 accumulation.
    Only the matrix entries carry bf16 rounding (~1.5e-3 rel).
  - Each chunk DMA moves one fully contiguous DRAM block; chunk c
    packs its subtiles' p columns followed by their stream columns.
    Chunk sizes (2,3,5x8,3,1 subtiles) start small so the PE gets data
    early and end small to shorten the final PE tail. Chunks
    0 and 3.. issue on the sync HWDGE ring (chunk 0 first so the PE's
    first data takes the shortest path); chunks 1-2 on the scalar ring
    so both rings push descriptors from t0 (high early demand ramps
    the DMA power state fast) and the tail chunks complete strictly
    in order on one ring.
  - Throwaway matmuls into a scratch PSUM bank before the first chunk
    hold the PE's DVFS ramp (0.65 -> 1.2 -> 2.4 GHz after ~3 us busy),
    so real matmuls run at full clock once data arrives.
  - The [2, R] PSUM result is evicted by the DVE and DMA'd from the
    warm sync ring; the out DMA carries the one allowed embedded
    semaphore wait on the eviction semaphore.
  - Host sums the 8 cores' [2, R] partials (all-reduce + hi/lo
    recombine) and scatters back to the [50, 2] diagrams.
"""

import numpy as np
import ml_dtypes

import concourse.bass as bass
import concourse.mybir as mybir
from concourse.bass_utils import run_bass_kernel_spmd

N_CORES = 8
P_FULL = 50000
H = W = 28
CARD = 50
NG = 2 * CARD  # gathered values per diagram = 100

K_PER = P_FULL // N_CORES  # 6250
SUB_PARTS = [128] * 48 + [112]  # k-subtile partition counts (sum 6256)
N_SUB = len(SUB_PARTS)  # 49
SUB_K0 = np.cumsum([0] + SUB_PARTS).tolist()  # k offset of each subtile
K_PAD = SUB_K0[-1]  # 6256 (last 6 rows zero)
CHUNK_SUBS = [2, 3, 5, 5, 5, 5, 5, 5, 5, 5, 3, 1]  # subtiles per chunk DMA
# mid chunks stay <= 5 subtiles: the ramped PE outruns chunk delivery
# by ~37 ns/subtile, and finer completion signals keep its catch-up
# waits short (long waits risk losing the PE clock ramp)
assert sum(CHUNK_SUBS) == N_SUB
N_CHUNK = len(CHUNK_SUBS)
CHUNK_S0 = np.cumsum([0] + CHUNK_SUBS).tolist()
SCALAR_CHUNKS = (1, 2)
SYNC_CHUNKS = tuple(c for c in range(N_CHUNK) if c not in SCALAR_CHUNKS)

BF16 = ml_dtypes.bfloat16
F32 = np.float32

# Throwaway matmuls into a scratch PSUM bank keep the tensor engine busy
# between the prologue and the first chunk's arrival: continuous
# activity holds the PE's DVFS ramp. Excess warmups are absorbed later
# (the ramped PE outruns the DMA stream), so generous is safe.
WARMUP_PRE = 22
# one filler matmul after each mid chunk keeps the PE busy across its
# short DMA waits so the DVFS ramp never resets mid-stream; at full
# clock the PE outruns chunk delivery, so fillers stay off the
# critical path
WARMUP_GAP = {c: 1 for c in range(2, 11)}


def build_nc(R: int) -> bass.Bass:
    """R = u1 + u2 streamed output columns per core."""
    f32 = mybir.dt.float32
    bf16 = mybir.dt.bfloat16
    nc = bass.Bass("TRN2")

    # chunk c: rows = sum of its subtiles' partitions, cols = 2 + R per
    # subtile (2 p-cols, then R stream cols, subtile-major)
    chunk_rows = [
        max(SUB_PARTS[CHUNK_S0[c] : CHUNK_S0[c + 1]]) for c in range(N_CHUNK)
    ]
    chunk_cols = [(2 + R) * CHUNK_SUBS[c] for c in range(N_CHUNK)]
    st_d = [
        nc.dram_tensor(f"st{c}", [chunk_rows[c], chunk_cols[c]], bf16,
                       kind="ExternalInput")
        for c in range(N_CHUNK)
    ]
    out_d = nc.dram_tensor("out", [2, R], f32, kind="ExternalOutput")
    st_cols = sum(chunk_cols)
    chunk_off = np.cumsum([0] + chunk_cols).tolist()

    from contextlib import ExitStack

    with ExitStack() as stk:
        st_sb = stk.enter_context(nc.sbuf_tensor("st_sb", [128, st_cols], bf16))
        out_sb = stk.enter_context(nc.sbuf_tensor("out_sb", [2, R], f32))
        ps = stk.enter_context(nc.psum_tensor("ps", [2, R], f32))
        ps_warm = stk.enter_context(nc.psum_tensor("ps_warm", [2, R], f32))

        ch_sems = [
            stk.enter_context(nc.semaphore(f"ch{q}")) for q in range(N_CHUNK)
        ]
        out_sem = stk.enter_context(nc.semaphore("out_sem"))
        pe_sem = stk.enter_context(nc.semaphore("pe_sem"))
        dve_sem = stk.enter_context(nc.semaphore("dve_sem"))
        block = stk.enter_context(nc.Block(no_gpsimd_drain=True))

        def chunk_dma(eng, c):
            cols = slice(chunk_off[c], chunk_off[c + 1])
            eng.dma_start(
                st_sb[0 : chunk_rows[c], cols], st_d[c][:, :]
            ).then_inc(ch_sems[c], 16)

        @block.sync
        def _(sync):
            for c in SYNC_CHUNKS:
                chunk_dma(sync, c)
            # output DMA on the same warm ring, with the one allowed
            # embedded wait on the eviction semaphore
            ins = sync.dma_start(out_d[:, :], out_sb[:, :]).then_inc(out_sem, 16)
            ins.wait_op(dve_sem, 1, "sem-ge")
            sync.wait_ge(out_sem, 16)

        @block.scalar
        def _(scalar):
            for c in SCALAR_CHUNKS:
                chunk_dma(scalar, c)

        @block.tensor
        def _(tensor):
            for _ in range(WARMUP_PRE):
                nc.tensor.matmul(
                    ps_warm[:, :], st_sb[:, 0:2], st_sb[:, 2 : 2 + R],
                    start=True, stop=True,
                )
            last = None
            s = 0
            for c in range(N_CHUNK):
                tensor.wait_ge(ch_sems[c], 16)
                off = chunk_off[c]
                n = CHUNK_SUBS[c]
                for j in range(n):
                    part = SUB_PARTS[s]
                    x_lo = off + 2 * n + j * R
                    last = nc.tensor.matmul(
                        ps[:, :],
                        st_sb[0:part, off + 2 * j : off + 2 * j + 2],
                        st_sb[0:part, x_lo : x_lo + R],
                        start=(s == 0),
                        stop=(s == N_SUB - 1),
                    )
                    s += 1
                for _ in range(WARMUP_GAP.get(c, 0)):
                    nc.tensor.matmul(
                        ps_warm[:, :], st_sb[:, 0:2], st_sb[:, 2 : 2 + R],
                        start=True, stop=True,
                    )
            last.then_inc(pe_sem, 1)

        @block.vector
        def _(vector):
            vector.wait_ge(pe_sem, 1)
            nc.vector.tensor_copy(out_sb[:, :], ps[:, :]).then_inc(dve_sem, 1)

    return nc


_NC_CACHE = {}


def get_nc(R: int) -> bass.Bass:
    if R not in _NC_CACHE:
        _NC_CACHE[R] = build_nc(R)
    return _NC_CACHE[R]


def shard_inputs(p, I, J, uniq1, uniq2) -> list[dict]:
    p = np.asarray(p, dtype=F32)
    u1, u2 = len(uniq1), len(uniq2)
    R = u1 + u2

    # Row gather on host (the "replicated trivially-small gather"), then
    # one bf16 cast of the [u, 50000] selections.
    I_sel = np.ascontiguousarray(np.asarray(I)[uniq1]).astype(BF16)
    J_sel = np.ascontiguousarray(np.asarray(J)[uniq2]).astype(BF16)

    in_maps = []
    for c in range(N_CORES):
        lo = c * K_PER
        hi = lo + K_PER

        pc = np.zeros(K_PAD, dtype=F32)
        pc[:K_PER] = p[lo:hi]
        phi = pc.astype(BF16)
        plo = (pc - phi.astype(F32)).astype(BF16)

        sel = np.zeros((R, K_PAD), dtype=BF16)
        sel[:u1, :K_PER] = I_sel[:, lo:hi]
        sel[u1:, :K_PER] = J_sel[:, lo:hi]

        im = {}
        for c2 in range(N_CHUNK):
            s_lo, s_hi = CHUNK_S0[c2], CHUNK_S0[c2 + 1]
            n = s_hi - s_lo
            rows = max(SUB_PARTS[s_lo:s_hi])
            blk = np.zeros((rows, (2 + R) * n), dtype=BF16)
            for j, s in enumerate(range(s_lo, s_hi)):
                part = SUB_PARTS[s]
                k0 = SUB_K0[s]
                blk[:part, 2 * j] = phi[k0 : k0 + part]
                blk[:part, 2 * j + 1] = plo[k0 : k0 + part]
                x_lo = 2 * n + j * R
                blk[:part, x_lo : x_lo + R] = sel[:, k0 : k0 + part].T
            im[f"st{c2}"] = blk
        in_maps.append(im)
    return in_maps


def run(p, I, J, inds1, inds2, trace=False, **run_kwargs):
    """Returns ((dgm1, dgm2), BassKernelResults)."""
    flat1 = np.asarray(inds1)[:, 0].astype(np.int64) * W + np.asarray(inds1)[:, 1]
    flat2 = np.asarray(inds2)[:, 0].astype(np.int64) * W + np.asarray(inds2)[:, 1]
    uniq1, inv1 = np.unique(flat1, return_inverse=True)
    uniq2, inv2 = np.unique(flat2, return_inverse=True)
    u1 = len(uniq1)
    R = u1 + len(uniq2)

    in_maps = shard_inputs(p, I, J, uniq1, uniq2)
    nc = get_nc(R)
    res = run_bass_kernel_spmd(
        nc, in_maps, list(range(N_CORES)), trace=trace, **run_kwargs
    )
    acc = np.zeros(R, dtype=np.float64)
    for r in res.results:
        o = r["out"].astype(np.float64)
        acc += o[0] + o[1]
    dgm1 = acc[inv1].astype(F32).reshape(-1, 2)
    dgm2 = acc[u1 + inv2].astype(F32).reshape(-1, 2)
    return (dgm1, dgm2), res


def kernel(p, I, J, inds1, inds2):
    out, _ = run(p, I, J, inds1, inds2, trace=False)
    return out
